# revision 1
# baseline (speedup 1.0000x reference)
"""Delay-and-sum (DAS) beamforming kernel for 8 Trainium2 NeuronCores.

Problem: out[b,p] = sum_d apod[d] * lerp(S[b,d], tof[p,d]) / sum(apod)
  with S = sino[b,0,d,:], lerp via floor index k0 and fraction alpha.

Sharding: data-parallel over pixels (8192 pixels per core); no collectives.

Per-core pipeline:
  - sino relaid out host-side as sg[d, t, b] (batch-minor) so one 32-byte
    indirect-DMA element per (pixel, detector) fetches both taps for all
    4 batches at once.
  - tof/alpha relaid detector-major [128, px] (partition = detector).
  - offsets = floor(tof) + 2048*d on DVE (HW cast is round-to-nearest, so
    floor = cast -> cast-back -> is_gt -> subtract).
  - SWDGE indirect gather -> G[d, (p, tap, b)].
  - DVE: R0 = G_tap0*(1-a), R1 = G_tap1*a (alpha broadcast over b, step-0 AP).
  - PE: psum[1,(p,b)] += apod^T @ R0 + apod^T @ R1 (reduce over detectors).
  - ACT evicts psum -> SBUF, HWDGE stores to HBM.
"""
import numpy as np

import concourse.bass as bass
import concourse.tile as tile
from concourse import bacc, mybir

N_DET, N_T, NY, NX, B = 128, 2048, 256, 256, 4
P_TOTAL = NY * NX
N_CORES = 8
PX_PER_CORE = P_TOTAL // N_CORES
CHUNK_PX = 512
F32 = mybir.dt.float32
I32 = mybir.dt.int32


def _build_kernel(px_per_core: int = PX_PER_CORE, chunk_px: int = CHUNK_PX):
    assert px_per_core % chunk_px == 0
    n_chunks = px_per_core // chunk_px

    nc = bacc.Bacc("TRN2", target_bir_lowering=False, debug=False)

    sg = nc.dram_tensor("sg", [N_DET * N_T, B], F32, kind="ExternalInput")
    tof_t = nc.dram_tensor("tof_t", [N_DET, px_per_core], F32, kind="ExternalInput")
    alpha_t = nc.dram_tensor("alpha_t", [N_DET, px_per_core], F32, kind="ExternalInput")
    apod = nc.dram_tensor("apod", [N_DET, 1], F32, kind="ExternalInput")
    dcol = nc.dram_tensor("dcol", [N_DET, 1], F32, kind="ExternalInput")
    outd = nc.dram_tensor("out", [n_chunks, chunk_px * B], F32, kind="ExternalOutput")

    n_q = (chunk_px * B + 511) // 512

    with tile.TileContext(nc) as tc:
        with (
            tc.tile_pool(name="const", bufs=1) as cpool,
            tc.tile_pool(name="io", bufs=3) as io,
            tc.tile_pool(name="idx", bufs=3) as idx,
            tc.tile_pool(name="gat", bufs=2) as gat,
            tc.tile_pool(name="rr", bufs=2) as rr,
            tc.tile_pool(name="ps", bufs=4, space="PSUM") as ps,
            tc.tile_pool(name="oc", bufs=3) as oc,
        ):
            apod_tl = cpool.tile([N_DET, 1], F32)
            nc.sync.dma_start(out=apod_tl[:], in_=apod.ap())
            dcol_tl = cpool.tile([N_DET, 1], F32)
            nc.sync.dma_start(out=dcol_tl[:], in_=dcol.ap())

            for c in range(n_chunks):
                sl = slice(c * chunk_px, (c + 1) * chunk_px)
                tof_tl = io.tile([N_DET, chunk_px], F32, tag="tof")
                nc.sync.dma_start(out=tof_tl[:], in_=tof_t.ap()[:, sl])
                alpha_tl = io.tile([N_DET, chunk_px], F32, tag="alpha")
                nc.sync.dma_start(out=alpha_tl[:], in_=alpha_t.ap()[:, sl])

                # floor(tof): round-to-nearest cast + correction
                r_i = idx.tile([N_DET, chunk_px], I32, tag="ri")
                nc.vector.tensor_copy(out=r_i[:], in_=tof_tl[:])
                r_f = idx.tile([N_DET, chunk_px], F32, tag="rf")
                nc.vector.tensor_copy(out=r_f[:], in_=r_i[:])
                m = idx.tile([N_DET, chunk_px], F32, tag="m")
                nc.vector.tensor_tensor(out=m[:], in0=r_f[:], in1=tof_tl[:],
                                        op=mybir.AluOpType.is_gt)
                k0f = idx.tile([N_DET, chunk_px], F32, tag="k0f")
                nc.vector.tensor_tensor(out=k0f[:], in0=r_f[:], in1=m[:],
                                        op=mybir.AluOpType.subtract)
                offs_f = idx.tile([N_DET, chunk_px], F32, tag="offsf")
                nc.vector.tensor_scalar_add(out=offs_f[:], in0=k0f[:],
                                            scalar1=dcol_tl[:])
                offs = idx.tile([N_DET, chunk_px], I32, tag="offs")
                nc.vector.tensor_copy(out=offs[:], in_=offs_f[:])

                # indirect gather: one instruction per pixel column; each moves
                # 128 rows (one per detector partition) of 8 f32 (s0*4b, s1*4b)
                G = gat.tile([N_DET, chunk_px * 8], F32, tag="G")
                for j in range(chunk_px):
                    nc.gpsimd.indirect_dma_start(
                        out=G[:, j * 8:(j + 1) * 8],
                        out_offset=None,
                        in_=sg.ap(),
                        in_offset=bass.IndirectOffsetOnAxis(
                            ap=offs[:, j:j + 1], axis=0),
                    )

                om_a = idx.tile([N_DET, chunk_px], F32, tag="oma")
                nc.vector.tensor_scalar(out=om_a[:], in0=alpha_tl[:],
                                        scalar1=-1.0, scalar2=1.0,
                                        op0=mybir.AluOpType.mult,
                                        op1=mybir.AluOpType.add)

                g_ap = G[:]
                part_dim = g_ap.ap[0]
                R0 = rr.tile([N_DET, chunk_px * B], F32, tag="R0")
                R1 = rr.tile([N_DET, chunk_px * B], F32, tag="R1")
                for tap, (w_tl, R) in enumerate(((om_a, R0), (alpha_tl, R1))):
                    g_tap = bass.AP(G.tensor, g_ap.offset + tap * 4,
                                    [part_dim, [8, chunk_px], [1, B]])
                    w_bc = bass.AP(w_tl.tensor, w_tl[:].offset,
                                   [w_tl[:].ap[0], [1, chunk_px], [0, B]])
                    nc.vector.tensor_tensor(
                        out=R[:].rearrange("d (p b) -> d p b", b=B),
                        in0=g_tap, in1=w_bc, op=mybir.AluOpType.mult)

                outc = oc.tile([1, chunk_px * B], F32, tag="outc")
                for q in range(n_q):
                    qs = slice(q * 512, min((q + 1) * 512, chunk_px * B))
                    n_cols = qs.stop - qs.start
                    psq = ps.tile([1, 512], F32, tag="psq")
                    nc.tensor.matmul(out=psq[:, :n_cols], lhsT=apod_tl[:],
                                     rhs=R0[:, qs], start=True, stop=False)
                    nc.tensor.matmul(out=psq[:, :n_cols], lhsT=apod_tl[:],
                                     rhs=R1[:, qs], start=False, stop=True)
                    nc.scalar.copy(out=outc[:1, qs], in_=psq[:, :n_cols])

                nc.sync.dma_start(out=outd.ap()[c:c + 1, :], in_=outc[:])

    nc.compile()
    return nc


def _host_prep(sino: np.ndarray, lut: np.ndarray, px_per_core: int = PX_PER_CORE):
    sino = np.ascontiguousarray(sino, dtype=np.float32)
    lut = np.ascontiguousarray(lut, dtype=np.float32)
    sg = np.ascontiguousarray(sino[:, 0].transpose(1, 2, 0)).reshape(N_DET * N_T, B)
    lut_flat = lut.reshape(P_TOTAL, N_DET, 2)
    tof_T = np.ascontiguousarray(lut_flat[:, :, 0].T)
    alpha_T = np.ascontiguousarray(lut_flat[:, :, 1].T)

    apod = (0.5 - 0.5 * np.cos(
        2.0 * np.pi * np.arange(N_DET, dtype=np.float32) / (N_DET - 1)
    )).astype(np.float32)
    norm = max(apod.sum(), np.finfo(np.float32).tiny)
    apod_n = (apod / norm).reshape(N_DET, 1).astype(np.float32)
    dcol = (np.arange(N_DET, dtype=np.float32) * N_T).reshape(N_DET, 1)

    n_cores = P_TOTAL // px_per_core
    in_maps = []
    for c in range(n_cores):
        sl = slice(c * px_per_core, (c + 1) * px_per_core)
        in_maps.append({
            "sg": sg,
            "tof_t": np.ascontiguousarray(tof_T[:, sl]),
            "alpha_t": np.ascontiguousarray(alpha_T[:, sl]),
            "apod": apod_n,
            "dcol": dcol,
        })
    return in_maps


def _assemble(results: list, px_per_core: int = PX_PER_CORE) -> np.ndarray:
    outs = [r["out"].reshape(px_per_core, B) for r in results]
    full = np.concatenate(outs, axis=0)  # [P_TOTAL, B]
    return np.ascontiguousarray(full.T).reshape(B, 1, NY, NX)


_CACHE: dict = {}


def _get_nc():
    if "nc" not in _CACHE:
        _CACHE["nc"] = _build_kernel()
    return _CACHE["nc"]


def kernel(sino: np.ndarray, lut: np.ndarray) -> np.ndarray:
    from concourse.bass_utils import run_bass_kernel_spmd

    nc = _get_nc()
    in_maps = _host_prep(np.asarray(sino), np.asarray(lut))
    res = run_bass_kernel_spmd(nc, in_maps, core_ids=list(range(N_CORES)))
    return _assemble(res.results)


def kernel_timed(inputs: dict, iters: int = 20) -> float:
    """Run the kernel repeatedly with device-resident inputs; return ns/iter."""
    import time
    import jax
    from jax.sharding import Mesh, PartitionSpec
    from jax.experimental.shard_map import shard_map
    from concourse.bass2jax import (
        _bass_exec_p, install_neuronx_cc_hook)
    import concourse.mybir as mybir_

    nc = _get_nc()
    in_maps = _host_prep(np.asarray(inputs["sino"]), np.asarray(inputs["lut"]))

    install_neuronx_cc_hook()
    part_name = nc.partition_id_tensor.name if nc.partition_id_tensor else None
    in_names, out_names, out_avals, zero_outs = [], [], [], []
    for alloc in nc.m.functions[0].allocations:
        if not isinstance(alloc, mybir_.MemoryLocationSet):
            continue
        name = alloc.memorylocations[0].name
        if alloc.kind == "ExternalInput":
            if name != part_name:
                in_names.append(name)
        elif alloc.kind == "ExternalOutput":
            out_names.append(name)
            shape = tuple(alloc.tensor_shape)
            dtype = mybir_.dt.np(alloc.dtype)
            out_avals.append(jax.core.ShapedArray(shape, dtype))
            zero_outs.append(np.zeros(shape, dtype))
    n_params = len(in_names)
    all_names = in_names + out_names
    if part_name is not None:
        all_names.append(part_name)
    from concourse.bass2jax import partition_id_tensor

    def _body(*args):
        operands = list(args)
        if part_name is not None:
            operands.append(partition_id_tensor())
        outs = _bass_exec_p.bind(
            *operands,
            out_avals=tuple(out_avals),
            in_names=tuple(all_names),
            out_names=tuple(out_names),
            lowering_input_output_aliases=(),
            sim_require_finite=True,
            sim_require_nnan=True,
            nc=nc,
        )
        return tuple(outs)

    devices = jax.devices()[:N_CORES]
    mesh = Mesh(np.asarray(devices), ("core",))
    n_outs = len(out_names)
    sharded = jax.jit(
        shard_map(_body, mesh=mesh,
                  in_specs=(PartitionSpec("core"),) * (n_params + n_outs),
                  out_specs=(PartitionSpec("core"),) * n_outs,
                  check_rep=False),
        keep_unused=True,
    )
    concat_in = [
        np.concatenate([in_maps[c][name] for c in range(N_CORES)], axis=0)
        for name in in_names
    ]
    concat_zeros = [
        np.zeros((N_CORES * z.shape[0], *z.shape[1:]), z.dtype) for z in zero_outs
    ]
    dev_in = [jax.device_put(a) for a in concat_in]
    dev_zero = [jax.device_put(a) for a in concat_zeros]

    # warmup (compile + 2 runs)
    for _ in range(3):
        outs = sharded(*dev_in, *dev_zero)
        jax.block_until_ready(outs)

    t0 = time.perf_counter()
    for _ in range(iters):
        outs = sharded(*dev_in, *dev_zero)
    jax.block_until_ready(outs)
    t1 = time.perf_counter()
    return (t1 - t0) / iters * 1e9



# revision 4
# speedup vs baseline: 796.8953x; 796.8953x over previous
"""Delay-and-sum (DAS) beamforming kernel for 8 Trainium2 NeuronCores.

Problem: out[b,p] = sum_d apod[d] * lerp(S[b,d], tof[p,d]) / sum(apod)
  with S = sino[b,0,d,:], lerp via floor index k0 and fraction alpha.

Sharding: data-parallel over pixels (8192 pixels per core); no collectives.

Per-core pipeline:
  - sino relaid out host-side as sg[d, t, b] (batch-minor) so one 32-byte
    indirect-DMA element per (pixel, detector) fetches both taps for all
    4 batches at once.
  - tof/alpha relaid detector-major [128, px] (partition = detector).
  - offsets = floor(tof) + 2048*d on DVE (HW cast is round-to-nearest, so
    floor = cast -> cast-back -> is_gt -> subtract).
  - SWDGE indirect gather -> G[d, (p, tap, b)].
  - DVE: R0 = G_tap0*(1-a), R1 = G_tap1*a (alpha broadcast over b, step-0 AP).
  - PE: psum[1,(p,b)] += apod^T @ R0 + apod^T @ R1 (reduce over detectors).
  - ACT evicts psum -> SBUF, HWDGE stores to HBM.
"""
import numpy as np

import concourse.bass as bass
import concourse.tile as tile
from concourse import bacc, mybir

N_DET, N_T, NY, NX, B = 128, 2048, 256, 256, 4
P_TOTAL = NY * NX
N_CORES = 8
PX_PER_CORE = P_TOTAL // N_CORES
CHUNK_PX = 512
F32 = mybir.dt.float32
BF16 = mybir.dt.bfloat16
I32 = mybir.dt.int32


def _build_kernel(px_per_core: int = PX_PER_CORE, chunk_px: int = CHUNK_PX):
    assert px_per_core % chunk_px == 0
    n_chunks = px_per_core // chunk_px

    nc = bacc.Bacc("TRN2", target_bir_lowering=False, debug=False)

    sg = nc.dram_tensor("sg", [N_DET * N_T, B], BF16, kind="ExternalInput")
    tof_t = nc.dram_tensor("tof_t", [N_DET, px_per_core], F32, kind="ExternalInput")
    alpha_t = nc.dram_tensor("alpha_t", [N_DET, px_per_core], F32, kind="ExternalInput")
    apod = nc.dram_tensor("apod", [N_DET, 1], F32, kind="ExternalInput")
    dcol = nc.dram_tensor("dcol", [N_DET, 1], F32, kind="ExternalInput")
    outd = nc.dram_tensor("out", [n_chunks, chunk_px * B], F32, kind="ExternalOutput")

    n_q = (chunk_px * B + 511) // 512

    with tile.TileContext(nc) as tc:
        with (
            tc.tile_pool(name="const", bufs=1) as cpool,
            tc.tile_pool(name="io", bufs=3) as io,
            tc.tile_pool(name="idx", bufs=3) as idx,
            tc.tile_pool(name="gat", bufs=3) as gat,
            tc.tile_pool(name="rr", bufs=2) as rr,
            tc.tile_pool(name="ps", bufs=4, space="PSUM") as ps,
            tc.tile_pool(name="oc", bufs=3) as oc,
        ):
            apod_tl = cpool.tile([N_DET, 1], F32)
            nc.sync.dma_start(out=apod_tl[:], in_=apod.ap())
            dcol_tl = cpool.tile([N_DET, 1], F32)
            nc.sync.dma_start(out=dcol_tl[:], in_=dcol.ap())

            for c in range(n_chunks):
                sl = slice(c * chunk_px, (c + 1) * chunk_px)
                tof_tl = io.tile([N_DET, chunk_px], F32, tag="tof")
                nc.sync.dma_start(out=tof_tl[:], in_=tof_t.ap()[:, sl])
                alpha_tl = io.tile([N_DET, chunk_px], F32, tag="alpha")
                nc.sync.dma_start(out=alpha_tl[:], in_=alpha_t.ap()[:, sl])

                # floor(tof): round-to-nearest cast + correction
                r_i = idx.tile([N_DET, chunk_px], I32, tag="ri")
                nc.vector.tensor_copy(out=r_i[:], in_=tof_tl[:])
                r_f = idx.tile([N_DET, chunk_px], F32, tag="rf")
                nc.vector.tensor_copy(out=r_f[:], in_=r_i[:])
                m = idx.tile([N_DET, chunk_px], F32, tag="m")
                nc.vector.tensor_tensor(out=m[:], in0=r_f[:], in1=tof_tl[:],
                                        op=mybir.AluOpType.is_gt)
                k0f = idx.tile([N_DET, chunk_px], F32, tag="k0f")
                nc.vector.tensor_tensor(out=k0f[:], in0=r_f[:], in1=m[:],
                                        op=mybir.AluOpType.subtract)
                offs_f = idx.tile([N_DET, chunk_px], F32, tag="offsf")
                nc.vector.tensor_scalar_add(out=offs_f[:], in0=k0f[:],
                                            scalar1=dcol_tl[:])
                offs = idx.tile([N_DET, chunk_px], I32, tag="offs")
                nc.vector.tensor_copy(out=offs[:], in_=offs_f[:])

                # indirect gather: one instruction per pixel column; each moves
                # 128 rows (one per detector partition) of 8 f32 (s0*4b, s1*4b)
                G = gat.tile([N_DET, chunk_px * 8], BF16, tag="G")
                for j in range(chunk_px):
                    nc.gpsimd.indirect_dma_start(
                        out=G[:, j * 8:(j + 1) * 8],
                        out_offset=None,
                        in_=sg.ap(),
                        in_offset=bass.IndirectOffsetOnAxis(
                            ap=offs[:, j:j + 1], axis=0),
                    )

                om_a = idx.tile([N_DET, chunk_px], F32, tag="oma")
                nc.vector.tensor_scalar(out=om_a[:], in0=alpha_tl[:],
                                        scalar1=-1.0, scalar2=1.0,
                                        op0=mybir.AluOpType.mult,
                                        op1=mybir.AluOpType.add)

                g_ap = G[:]
                part_dim = g_ap.ap[0]
                R0 = rr.tile([N_DET, chunk_px * B], F32, tag="R0")
                R1 = rr.tile([N_DET, chunk_px * B], F32, tag="R1")
                for tap, (w_tl, R) in enumerate(((om_a, R0), (alpha_tl, R1))):
                    g_tap = bass.AP(G.tensor, g_ap.offset + tap * 4,
                                    [part_dim, [8, chunk_px], [1, B]])
                    w_bc = bass.AP(w_tl.tensor, w_tl[:].offset,
                                   [w_tl[:].ap[0], [1, chunk_px], [0, B]])
                    nc.vector.tensor_tensor(
                        out=R[:].rearrange("d (p b) -> d p b", b=B),
                        in0=g_tap, in1=w_bc, op=mybir.AluOpType.mult)

                outc = oc.tile([1, chunk_px * B], F32, tag="outc")
                for q in range(n_q):
                    qs = slice(q * 512, min((q + 1) * 512, chunk_px * B))
                    n_cols = qs.stop - qs.start
                    psq = ps.tile([1, 512], F32, tag="psq")
                    nc.tensor.matmul(out=psq[:, :n_cols], lhsT=apod_tl[:],
                                     rhs=R0[:, qs], start=True, stop=False)
                    nc.tensor.matmul(out=psq[:, :n_cols], lhsT=apod_tl[:],
                                     rhs=R1[:, qs], start=False, stop=True)
                    nc.scalar.copy(out=outc[:1, qs], in_=psq[:, :n_cols])

                nc.sync.dma_start(out=outd.ap()[c:c + 1, :], in_=outc[:])

    nc.compile()
    return nc


def _host_prep(sino: np.ndarray, lut: np.ndarray, px_per_core: int = PX_PER_CORE):
    sino = np.ascontiguousarray(sino, dtype=np.float32)
    lut = np.ascontiguousarray(lut, dtype=np.float32)
    import ml_dtypes
    sg = np.ascontiguousarray(sino[:, 0].transpose(1, 2, 0)).reshape(
        N_DET * N_T, B).astype(ml_dtypes.bfloat16)
    lut_flat = lut.reshape(P_TOTAL, N_DET, 2)
    tof_T = np.ascontiguousarray(lut_flat[:, :, 0].T)
    alpha_T = np.ascontiguousarray(lut_flat[:, :, 1].T)

    apod = (0.5 - 0.5 * np.cos(
        2.0 * np.pi * np.arange(N_DET, dtype=np.float32) / (N_DET - 1)
    )).astype(np.float32)
    norm = max(apod.sum(), np.finfo(np.float32).tiny)
    apod_n = (apod / norm).reshape(N_DET, 1).astype(np.float32)
    dcol = (np.arange(N_DET, dtype=np.float32) * N_T).reshape(N_DET, 1)

    n_cores = P_TOTAL // px_per_core
    in_maps = []
    for c in range(n_cores):
        sl = slice(c * px_per_core, (c + 1) * px_per_core)
        in_maps.append({
            "sg": sg,
            "tof_t": np.ascontiguousarray(tof_T[:, sl]),
            "alpha_t": np.ascontiguousarray(alpha_T[:, sl]),
            "apod": apod_n,
            "dcol": dcol,
        })
    return in_maps


def _assemble(results: list, px_per_core: int = PX_PER_CORE) -> np.ndarray:
    outs = [r["out"].reshape(px_per_core, B) for r in results]
    full = np.concatenate(outs, axis=0)  # [P_TOTAL, B]
    return np.ascontiguousarray(full.T).reshape(B, 1, NY, NX)


_CACHE: dict = {}


def _get_nc():
    if "nc" not in _CACHE:
        _CACHE["nc"] = _build_kernel()
    return _CACHE["nc"]


def kernel(sino: np.ndarray, lut: np.ndarray) -> np.ndarray:
    from concourse.bass_utils import run_bass_kernel_spmd

    nc = _get_nc()
    in_maps = _host_prep(np.asarray(sino), np.asarray(lut))
    res = run_bass_kernel_spmd(nc, in_maps, core_ids=list(range(N_CORES)))
    return _assemble(res.results)


def kernel_timed(inputs: dict, iters: int = 20) -> float:
    """Run the kernel repeatedly with device-resident inputs; return ns/iter."""
    import time
    import jax
    from jax.sharding import Mesh, PartitionSpec
    from jax.experimental.shard_map import shard_map
    from concourse.bass2jax import (
        _bass_exec_p, install_neuronx_cc_hook)
    import concourse.mybir as mybir_

    nc = _get_nc()
    in_maps = _host_prep(np.asarray(inputs["sino"]), np.asarray(inputs["lut"]))

    install_neuronx_cc_hook()
    part_name = nc.partition_id_tensor.name if nc.partition_id_tensor else None
    in_names, out_names, out_avals, zero_outs = [], [], [], []
    for alloc in nc.m.functions[0].allocations:
        if not isinstance(alloc, mybir_.MemoryLocationSet):
            continue
        name = alloc.memorylocations[0].name
        if alloc.kind == "ExternalInput":
            if name != part_name:
                in_names.append(name)
        elif alloc.kind == "ExternalOutput":
            out_names.append(name)
            shape = tuple(alloc.tensor_shape)
            dtype = mybir_.dt.np(alloc.dtype)
            out_avals.append(jax.core.ShapedArray(shape, dtype))
            zero_outs.append(np.zeros(shape, dtype))
    n_params = len(in_names)
    all_names = in_names + out_names
    if part_name is not None:
        all_names.append(part_name)
    from concourse.bass2jax import partition_id_tensor

    def _body(*args):
        operands = list(args)
        if part_name is not None:
            operands.append(partition_id_tensor())
        outs = _bass_exec_p.bind(
            *operands,
            out_avals=tuple(out_avals),
            in_names=tuple(all_names),
            out_names=tuple(out_names),
            lowering_input_output_aliases=(),
            sim_require_finite=True,
            sim_require_nnan=True,
            nc=nc,
        )
        return tuple(outs)

    devices = jax.devices()[:N_CORES]
    mesh = Mesh(np.asarray(devices), ("core",))
    n_outs = len(out_names)
    sharded = jax.jit(
        shard_map(_body, mesh=mesh,
                  in_specs=(PartitionSpec("core"),) * (n_params + n_outs),
                  out_specs=(PartitionSpec("core"),) * n_outs,
                  check_rep=False),
        keep_unused=True,
    )
    concat_in = [
        np.concatenate([in_maps[c][name] for c in range(N_CORES)], axis=0)
        for name in in_names
    ]
    concat_zeros = [
        np.zeros((N_CORES * z.shape[0], *z.shape[1:]), z.dtype) for z in zero_outs
    ]
    dev_in = [jax.device_put(a) for a in concat_in]
    dev_zero = [jax.device_put(a) for a in concat_zeros]

    # warmup (compile + 2 runs)
    for _ in range(3):
        outs = sharded(*dev_in, *dev_zero)
        jax.block_until_ready(outs)

    t0 = time.perf_counter()
    for _ in range(iters):
        outs = sharded(*dev_in, *dev_zero)
    jax.block_until_ready(outs)
    t1 = time.perf_counter()
    return (t1 - t0) / iters * 1e9

